# revision 1
# baseline (speedup 1.0000x reference)
"""DecoderLSTM w/ Bahdanau attention for trn2, 8 NeuronCores, data-parallel over batch.

Sharding: batch B=32 -> 4 samples/core, zero collectives.
Phase 0: SBUF-load weights/enc, PE-transpose enc, enc_proj in [A-part,(b,s)] (bf16).
Phase 1: 99 sequential steps fully SBUF-resident:
  dec_proj.T (weight-stationary f32r) -> tanh w/ per-partition bias (ACT, bf16 out)
  scores = v.T @ T (tile-position packed, psum rows {0,32,64,96})
  exp w/o max-subtraction (|s|<=8.2) + accum sums; alpha.T via PE transposes
  context (packed matvecs) scaled 1/sum on ACT copy; gates via f32r weights-moving
  [4,2048] -> ACT/DVE copy -> PE transpose -> [128,(gc,b)]; sigmoid via tanh
  (single ACT table: exp_and_others = {exp, tanh, copy, identity}).
Phase 2: logits = h_hist @ fc_W.T batched over 99 steps (bf16, streamed weights).
"""

import numpy as np
import ml_dtypes

import concourse.bass as bass
import concourse.bacc as bacc
import concourse.mybir as mybir
import concourse.tile as tile
from concourse.bass_utils import run_bass_kernel_spmd

B, S, T = 32, 400, 100
E, H, A, V = 256, 512, 512, 32000
G = 4 * H
NC = 8
BL = B // NC          # 4 local samples
TS = T - 1            # 99 steps
F32 = mybir.dt.float32
F32R = mybir.dt.float32r
BF16 = mybir.dt.bfloat16
AF = mybir.ActivationFunctionType

HC = H // 128   # 4
ACk = A // 128  # 4
ECk = E // 128  # 2
KC = ECk + 2 * HC  # 10 gate K chunks (emb, ctx, h)
GC = G // 128   # 16
S_CHUNKS = [(i * 128, min(128, S - i * 128)) for i in range((S + 127) // 128)]


def ceil_div(a, b):
    return (a + b - 1) // b


def build_nc(n_steps=TS, with_fcb=False):
    nc = bacc.Bacc()
    ts_rows = n_steps * BL

    d_enc = nc.dram_tensor("enc", [BL, S, H], BF16, kind="ExternalInput")
    d_h0T = nc.dram_tensor("h0T", [H, BL], F32R, kind="ExternalInput")
    d_c0t = nc.dram_tensor("c0t", [128, 4 * BL], F32, kind="ExternalInput")
    d_embT = nc.dram_tensor("embT", [E, TS * BL], F32R, kind="ExternalInput")
    d_WgT = nc.dram_tensor("WgT", [E + 2 * H, G], F32R, kind="ExternalInput")
    d_Wdec = nc.dram_tensor("Wdec", [H, A], F32R, kind="ExternalInput")
    d_Wenc = nc.dram_tensor("Wenc", [H, A], BF16, kind="ExternalInput")
    d_v = nc.dram_tensor("vt", [128, 4], BF16, kind="ExternalInput")
    d_bdec = nc.dram_tensor("bdec_t", [128, 16], F32, kind="ExternalInput")
    d_bg = nc.dram_tensor("bg_t", [128, 64], F32, kind="ExternalInput")
    d_ident = nc.dram_tensor("ident", [128, 128], F32, kind="ExternalInput")
    d_id16 = nc.dram_tensor("id16", [128, 128], BF16, kind="ExternalInput")
    d_fcWT = nc.dram_tensor("fcWT", [H, V], BF16, kind="ExternalInput")
    d_fcb = nc.dram_tensor("fcb", [1, V], BF16, kind="ExternalInput")
    d_out = nc.dram_tensor("out", [ts_rows, V], F32, kind="ExternalOutput")

    with tile.TileContext(nc) as tc:
        import contextlib
        stack = contextlib.ExitStack()
        with stack:
            P = lambda name, bufs, space="SBUF": stack.enter_context(
                tc.tile_pool(name=name, bufs=bufs, space=space))
            singles = P("singles", 1)
            trans = P("trans", 2)    # transient sbuf (Wenc, fw tiles share)
            st = P("st", 2)          # small per-step sbuf scratch
            stg = P("stg", 1)        # [4, 2048] gates sbuf
            stc = P("stc", 2)        # c state ping-pong
            Tp = P("Tp", 1)          # big [128, BL*S] tiles (encT/T)
            ob = P("ob", 2)          # phase2 out staging
            # PSUM: exactly 8 banks
            p_small = P("p_small", 2, space="PSUM")   # tag psm, <=512B
            p_sc = P("p_sc", 1, space="PSUM")         # tag sc, [128,400]
            p_cx = P("p_cx", 1, space="PSUM")         # tag cx, [128,512]
            p_g = P("p_g", 1, space="PSUM")           # tags pg0..3, 2KB each

            # ---- persistent SBUF ----
            sb_enc = [[singles.tile([sp, H], BF16, tag=f"enc_{b}_{ci}", name=f"enc_{b}_{ci}")
                       for ci, (so, sp) in enumerate(S_CHUNKS)] for b in range(BL)]
            for b in range(BL):
                for ci, (so, sp) in enumerate(S_CHUNKS):
                    nc.sync.dma_start(out=sb_enc[b][ci], in_=d_enc[b, so:so + sp, :])
            sb_WgT = [singles.tile([128, G], F32R, tag=f"wg_{k}", name=f"wg_{k}") for k in range(KC)]
            for k in range(KC):
                nc.sync.dma_start(out=sb_WgT[k], in_=d_WgT[k * 128:(k + 1) * 128, :])
            sb_Wdec = [singles.tile([128, A], F32R, tag=f"wd_{k}", name=f"wd_{k}") for k in range(HC)]
            for k in range(HC):
                nc.sync.dma_start(out=sb_Wdec[k], in_=d_Wdec[k * 128:(k + 1) * 128, :])
            sb_v = singles.tile([128, 4], BF16)
            nc.sync.dma_start(out=sb_v, in_=d_v[:, :])
            sb_bdec = singles.tile([128, 16], F32)
            nc.sync.dma_start(out=sb_bdec, in_=d_bdec[:, :])
            sb_bg = singles.tile([128, 64], F32)
            nc.sync.dma_start(out=sb_bg, in_=d_bg[:, :])
            sb_id = singles.tile([128, 128], F32)
            nc.sync.dma_start(out=sb_id, in_=d_ident[:, :])
            sb_id16 = singles.tile([128, 128], BF16)
            nc.sync.dma_start(out=sb_id16, in_=d_id16[:, :])
            sb_ones1 = singles.tile([128, 1], F32)
            nc.vector.memset(sb_ones1, 1.0)
            sb_h = [singles.tile([128, (TS + 1) * BL], F32R, tag=f"hh_{k}", name=f"hh_{k}")
                    for k in range(HC)]
            for k in range(HC):
                nc.sync.dma_start(out=sb_h[k][:, 0:BL],
                                  in_=d_h0T[k * 128:(k + 1) * 128, :])
            sb_embT = [singles.tile([128, TS * BL], F32R, tag=f"em_{k}", name=f"em_{k}")
                       for k in range(ECk)]
            for k in range(ECk):
                nc.sync.dma_start(out=sb_embT[k], in_=d_embT[k * 128:(k + 1) * 128, :])
            sb_encp = [singles.tile([128, BL * S], BF16, tag=f"ep_{a}", name=f"ep_{a}")
                       for a in range(ACk)]

            # ---- phase 0: enc transpose + enc_proj ----
            sb_Wenc = [trans.tile([128, A], BF16, tag=f"fw_{k}", name=f"wenc_{k}") for k in range(HC)]
            for k in range(HC):
                nc.sync.dma_start(out=sb_Wenc[k], in_=d_Wenc[k * 128:(k + 1) * 128, :])
            sb_encT = [Tp.tile([128, BL * S], BF16, tag=f"tt_{k}", name=f"encT_{k}") for k in range(HC)]
            for b in range(BL):
                for ci, (so, sp) in enumerate(S_CHUNKS):
                    for k in range(HC):
                        ptr = p_small.tile([128, 128], BF16, tag="psm")
                        nc.tensor.transpose(
                            ptr[:, 0:sp], sb_enc[b][ci][:, k * 128:(k + 1) * 128],
                            sb_id16[0:sp, 0:sp])
                        nc.vector.tensor_copy(
                            sb_encT[k][:, b * S + so: b * S + so + sp], ptr[:, 0:sp])
            for a in range(ACk):
                for b in range(BL):
                    pep = p_sc.tile([128, S], F32, tag="sc")
                    for k in range(HC):
                        nc.tensor.matmul(
                            pep,
                            sb_Wenc[k][:, a * 128:(a + 1) * 128],
                            sb_encT[k][:, b * S:(b + 1) * S],
                            start=(k == 0), stop=(k == HC - 1))
                    if (a * BL + b) % 2 == 0:
                        nc.scalar.copy(sb_encp[a][:, b * S:(b + 1) * S], pep)
                    else:
                        nc.vector.tensor_copy(sb_encp[a][:, b * S:(b + 1) * S], pep)

            # ---- phase 1: recurrence ----
            sb_c = stc.tile([128, 4 * BL], F32, tag="cst")
            nc.sync.dma_start(out=sb_c, in_=d_c0t[:, :])

            for t in range(n_steps):
                hcol = slice(t * BL, (t + 1) * BL)
                # dec_proj.T [A-part, (ac,b)] weight-stationary f32r
                pdT = p_small.tile([128, 4 * BL], F32, tag="psm")
                for a in range(ACk):
                    for k in range(HC):
                        nc.tensor.matmul(
                            pdT[:, a * BL:(a + 1) * BL],
                            sb_Wdec[k][:, a * 128:(a + 1) * 128],
                            sb_h[k][:, hcol],
                            start=(k == 0), stop=(k == HC - 1))
                s_dT = st.tile([128, 4 * BL], F32, tag="dT")
                nc.vector.tensor_add(s_dT, pdT, sb_bdec)

                # gates: emb + h parts early (overlaps the tanh below)
                pg = [p_g.tile([BL, 512], F32, tag=f"pg{gq}", name=f"pg{gq}") for gq in range(4)]
                for gq in range(4):
                    for k in range(ECk):
                        nc.tensor.matmul(
                            pg[gq],
                            sb_embT[k][:, hcol],
                            sb_WgT[k][:, gq * 512:(gq + 1) * 512],
                            start=(k == 0), stop=False, skip_group_check=True)
                    for k in range(HC):
                        nc.tensor.matmul(
                            pg[gq],
                            sb_h[k][:, hcol],
                            sb_WgT[ECk + HC + k][:, gq * 512:(gq + 1) * 512],
                            start=False, stop=False, skip_group_check=True)

                # T = tanh(encp + decp) [A-part, (b,s)] bf16
                sb_T = [Tp.tile([128, BL * S], BF16, tag=f"tt_{a}", name=f"T_{a}")
                        for a in range(ACk)]
                for a in range(ACk):
                    for b in range(BL):
                        nc.scalar.activation(
                            sb_T[a][:, b * S:(b + 1) * S],
                            sb_encp[a][:, b * S:(b + 1) * S],
                            AF.Tanh, bias=s_dT[:, a * BL + b: a * BL + b + 1])

                # scores (4-way col-packed)
                psc = p_sc.tile([128, S], F32, tag="sc")
                for a in range(ACk):
                    for b in range(BL):
                        nc.tensor.matmul(
                            psc[32 * b:32 * b + 1, :],
                            sb_v[:, a:a + 1],
                            sb_T[a][:, b * S:(b + 1) * S],
                            start=(a == 0), stop=(a == ACk - 1),
                            tile_position=(0, 32 * b), skip_group_check=True)

                s_exp = st.tile([128, S], F32, tag="exp")
                s_sum = st.tile([128, 1], F32, tag="sum")
                s_inv = st.tile([128, 1], F32, tag="inv")
                for b in range(BL):
                    nc.scalar.activation(
                        s_exp[32 * b:32 * b + 1, :], psc[32 * b:32 * b + 1, :],
                        AF.Exp, accum_out=s_sum[32 * b:32 * b + 1, 0:1])
                    nc.vector.reciprocal(s_inv[32 * b:32 * b + 1, 0:1],
                                         s_sum[32 * b:32 * b + 1, 0:1])

                # alpha.T -> [S-part, (sc,b)] bf16
                paT = p_small.tile([128, 4 * BL], F32, tag="psm")
                for ci, (so, sp) in enumerate(S_CHUNKS):
                    for b in range(BL):
                        nc.tensor.matmul(
                            paT[0:sp, ci * BL + b:ci * BL + b + 1],
                            s_exp[32 * b:32 * b + 1, so:so + sp],
                            sb_ones1[32 * b:32 * b + 1, 0:1],
                            tile_position=(32 * b, 0))
                s_aT = st.tile([128, 4 * BL], BF16, tag="aT")
                nc.vector.tensor_copy(s_aT, paT)

                # context (4-way col-packed), scale 1/sum on copy
                pcx = p_cx.tile([128, H], F32, tag="cx")
                for ci, (so, sp) in enumerate(S_CHUNKS):
                    for b in range(BL):
                        nc.tensor.matmul(
                            pcx[32 * b:32 * b + 1, :],
                            s_aT[0:sp, ci * BL + b:ci * BL + b + 1],
                            sb_enc[b][ci][0:sp, :],
                            start=(ci == 0), stop=(ci == len(S_CHUNKS) - 1),
                            tile_position=(0, 32 * b), skip_group_check=True)
                s_cx = st.tile([128, H], F32, tag="cx")
                for b in range(BL):
                    dst = s_cx[32 * b:32 * b + 1, :]
                    srcp = pcx[32 * b:32 * b + 1, :]
                    if b % 2 == 0:
                        nc.vector.tensor_copy(dst, srcp)
                    else:
                        nc.scalar.copy(dst, srcp)

                # ctx.T -> [H-part, (hc,b)] f32
                pcT = p_small.tile([128, 4 * BL], F32, tag="psm")
                for k in range(HC):
                    for b in range(BL):
                        nc.tensor.matmul(
                            pcT[:, k * BL + b:k * BL + b + 1],
                            s_cx[32 * b:32 * b + 1, k * 128:(k + 1) * 128],
                            s_inv[32 * b:32 * b + 1, 0:1],
                            tile_position=(32 * b, 0))
                s_cT = st.tile([128, 4 * BL], F32R, tag="cT")
                nc.vector.tensor_copy(s_cT, pcT)

                # gates ctx part (finishes accumulation)
                for gq in range(4):
                    for k in range(HC):
                        nc.tensor.matmul(
                            pg[gq],
                            s_cT[:, k * BL:(k + 1) * BL],
                            sb_WgT[ECk + k][:, gq * 512:(gq + 1) * 512],
                            start=False, stop=(k == HC - 1), skip_group_check=True)

                # [4,2048] psum -> sbuf (split engines), transpose to [128,(gc,b)]
                s_g = stg.tile([BL, G], F32, tag="g")
                for gq in range(4):
                    dst = s_g[:, gq * 512:(gq + 1) * 512]
                    if gq % 2 == 0:
                        nc.scalar.copy(dst, pg[gq])
                    else:
                        nc.vector.tensor_copy(dst, pg[gq])
                pgT = p_small.tile([128, 4 * GC], F32, tag="psm")
                for gc in range(GC):
                    nc.tensor.transpose(
                        pgT[:, gc * BL:(gc + 1) * BL],
                        s_g[0:BL, gc * 128:(gc + 1) * 128],
                        sb_id[0:BL, 0:BL])
                s_gb = st.tile([128, 4 * GC], F32, tag="gb")
                nc.vector.tensor_add(s_gb, pgT, sb_bg)

                # i,f: cols 0:32 | g: 32:48 | o: 48:64
                s_act = st.tile([128, 4 * GC], F32, tag="gact")
                nc.scalar.activation(s_act[:, 0:32], s_gb[:, 0:32], AF.Tanh, scale=0.5)
                nc.scalar.activation(s_act[:, 48:64], s_gb[:, 48:64], AF.Tanh,
                                     scale=0.5)
                nc.scalar.activation(s_act[:, 32:48], s_gb[:, 32:48], AF.Tanh)

                ti, tf = s_act[:, 0:16], s_act[:, 16:32]
                tg, to = s_act[:, 32:48], s_act[:, 48:64]
                u1 = st.tile([128, 16], F32, tag="u1")
                u2 = st.tile([128, 16], F32, tag="u2")
                nc.vector.tensor_mul(u1, tf, sb_c)
                nc.vector.tensor_add(u1, u1, sb_c)
                nc.vector.tensor_mul(u2, ti, tg)
                nc.vector.tensor_add(u2, u2, tg)
                c_new = stc.tile([128, 4 * BL], F32, tag="cst")
                nc.vector.tensor_add(c_new, u1, u2)
                nc.vector.tensor_scalar_mul(c_new, c_new, 0.5)
                sb_c = c_new
                tc_t = st.tile([128, 16], F32, tag="tc")
                nc.scalar.activation(tc_t, c_new, AF.Tanh)
                u3 = st.tile([128, 16], F32, tag="u3")
                nc.vector.tensor_mul(u3, to, tc_t)
                nc.vector.tensor_add(u3, u3, tc_t)
                for k in range(HC):
                    nc.vector.tensor_scalar_mul(
                        sb_h[k][:, (t + 1) * BL:(t + 2) * BL],
                        u3[:, k * BL:(k + 1) * BL], 0.5)

            # ---- phase 2: logits = h_hist @ fc_W.T (+fc_b) ----
            h16 = [singles.tile([128, ts_rows], BF16, tag=f"h16_{k}", name=f"h16_{k}")
                   for k in range(HC)]
            for k in range(HC):
                nc.vector.tensor_copy(h16[k], sb_h[k][:, BL:(n_steps + 1) * BL])
            n_chunks = [(i * 512, min(512, V - i * 512))
                        for i in range(ceil_div(V, 512))]
            m_chunks = [(i * 128, min(128, ts_rows - i * 128))
                        for i in range(ceil_div(ts_rows, 128))]
            if with_fcb:
                ones_sb = singles.tile([1, 128], BF16)
                nc.vector.memset(ones_sb, 1.0)
                fcb_sb = singles.tile([1, V], BF16)
                nc.sync.dma_start(out=fcb_sb, in_=d_fcb[:, :])
            for ni, (no, nn) in enumerate(n_chunks):
                fwt = [trans.tile([128, 512], BF16, tag=f"fw_{k}", name=f"fwt_{k}") for k in range(HC)]
                for k in range(HC):
                    nc.sync.dma_start(out=fwt[k][:, 0:nn],
                                      in_=d_fcWT[k * 128:(k + 1) * 128, no:no + nn])
                for mi, (mo, mp) in enumerate(m_chunks):
                    pot = p_g.tile([128, 512], F32, tag=f"pg{mi % 4}", name=f"pot{mi % 4}")
                    for k in range(HC):
                        nc.tensor.matmul(
                            pot[0:mp, 0:nn], h16[k][:, mo:mo + mp], fwt[k][:, 0:nn],
                            start=(k == 0), stop=(k == HC - 1 and not with_fcb))
                    if with_fcb:
                        nc.tensor.matmul(
                            pot[0:mp, 0:nn], ones_sb[:, 0:mp],
                            fcb_sb[:, no:no + nn],
                            start=False, stop=True, skip_group_check=True)
                    obt = ob.tile([128, 512], F32, tag="ob")
                    if mi % 2 == 0:
                        nc.vector.tensor_copy(obt[0:mp, 0:nn], pot[0:mp, 0:nn])
                    else:
                        nc.scalar.copy(obt[0:mp, 0:nn], pot[0:mp, 0:nn])
                    nc.gpsimd.dma_start(out=d_out[mo:mo + mp, no:no + nn],
                                        in_=obt[0:mp, 0:nn])
    nc.finalize()
    return nc


def _prep_inputs(encoder_outputs, hidden0, cell0, summary, summary_len,
                 embedding, W_ih, b_ih, W_hh, b_hh,
                 att_Wenc, att_benc, att_Wdec, att_bdec, att_v,
                 fc_W, fc_b):
    f32 = np.float32
    bf16 = ml_dtypes.bfloat16
    summary = np.asarray(summary)
    sm = summary.copy()
    sm[np.arange(B), np.asarray(summary_len) - 1] = 0
    sm = sm[:, :-1]                                  # [B, 99]
    emb = np.asarray(embedding, f32)[sm]             # [B, 99, E]

    WgT = np.ascontiguousarray(np.concatenate(
        [np.asarray(W_ih, f32).T, np.asarray(W_hh, f32).T], axis=0))  # [E+2H, G]
    bg = np.asarray(b_ih, f32) + np.asarray(b_hh, f32)
    bg_t = np.ascontiguousarray(
        np.repeat(bg.reshape(16, 128).T[:, :, None], BL, axis=2).reshape(128, 64))
    bd = np.asarray(att_benc, f32) + np.asarray(att_bdec, f32)
    bd_t = np.ascontiguousarray(
        np.repeat(bd.reshape(4, 128).T[:, :, None], BL, axis=2).reshape(128, 16))
    vt = np.ascontiguousarray(np.asarray(att_v, f32).reshape(4, 128).T).astype(bf16)
    ident = np.eye(128, dtype=f32)
    fcWT = np.ascontiguousarray(np.asarray(fc_W, f32).T).astype(bf16)
    fcb = np.ascontiguousarray(np.asarray(fc_b, f32).reshape(1, V)).astype(bf16)
    with_fcb = bool(np.any(np.asarray(fc_b)))

    h0 = np.asarray(hidden0, f32)
    c0 = np.asarray(cell0, f32)
    enc = np.asarray(encoder_outputs, f32)

    in_maps = []
    for c in range(NC):
        bs = slice(c * BL, (c + 1) * BL)
        h0T = np.ascontiguousarray(h0[bs].T)
        c0t = np.ascontiguousarray(
            c0[bs].T.reshape(4, 128, BL).transpose(1, 0, 2).reshape(128, 4 * BL))
        embT = np.ascontiguousarray(
            emb[bs].transpose(2, 1, 0).reshape(E, TS * BL))
        in_maps.append({
            "enc": np.ascontiguousarray(enc[bs]).astype(bf16),
            "h0T": h0T, "c0t": c0t, "embT": embT,
            "WgT": WgT,
            "Wdec": np.ascontiguousarray(np.asarray(att_Wdec, f32)),
            "Wenc": np.ascontiguousarray(np.asarray(att_Wenc, f32)).astype(bf16),
            "vt": vt, "bdec_t": bd_t, "bg_t": bg_t,
            "ident": ident, "id16": ident.astype(bf16),
            "fcWT": fcWT, "fcb": fcb,
        })
    return in_maps, with_fcb


_NC_CACHE = {}


def kernel(**inputs):
    in_maps, with_fcb = _prep_inputs(**inputs)
    key = (TS, with_fcb)
    if key not in _NC_CACHE:
        _NC_CACHE[key] = build_nc(TS, with_fcb)
    nc = _NC_CACHE[key]
    res = run_bass_kernel_spmd(nc, in_maps, list(range(NC)))
    outs = []
    for c in range(NC):
        o = np.asarray(res.results[c]["out"])        # [(t,b), V]
        outs.append(o.reshape(TS, BL, V).transpose(1, 0, 2))
    return np.concatenate(outs, axis=0).astype(np.float32)

